# revision 47
# baseline (speedup 1.0000x reference)
"""Cross multi-head attention on 8 Trainium2 NeuronCores.

Sharding: tensor-parallel by heads x data-parallel by batch. Core c
handles batch b = c//2 and head-half j = c%2 (heads 8j..8j+7). Each core
computes its heads' Q/K/V over the full sequence, attention, and a
PARTIAL output projection (contraction over its 512 concat dims); the
host sums the two partials per batch and adds bo. No K/V recompute, no
collectives.

Per-core kernel (T-major layout; host pre-transposes x/y, packs weights
head-pair-interleaved and hp-outermost so every weight DMA is
contiguous). Optimizations over the 409us baseline (~395us now):
  - K bias dropped entirely (it only adds a per-q constant to scores ->
    softmax-invariant); V bias folded into host-side
    bo_eff = bo + bv @ Wo (softmax weights sum to 1, so the v-bias
    contributes exactly bv@Wo) -- kills the 16 K=1 bias matmuls
  - input DMA on the sync+gpsimd rings ONLY: DMA descriptor issue on
    the scalar engine (= ACT) would block the exp stream, which paces
    the whole kernel (256 x [128,1024] exp activations ~278us busy)
  - t-chunk-major y DMA + per-tc K(hp0) + interleaved V-lead so the PE
    starts dense, HAM-warm projection work on the first quarter of y;
    V 10..15, Q(hp0) c1..3 finish inside the first attention unit
  - per (head-pair, query-chunk) unit: scoresT = KT_h.T @ QT_h as K=64
    row pairs on complementary PE halves (co-execute, ~219ns/pair
    measured vs 434 serial); oT' += V'_h.T @ expT stays M=65 serial
    (64 d + a ones column accumulating the softmax denominator in PSUM
    row 64; 2x65 output columns cannot col-pair within 128 lanes)
  - attnV lags exp by TWO tts so the PE never parks at the FIFO head
    waiting on ACT; Q/K(hp+1) and out-projection matmuls are emitted
    between attention groups as rate-controlled filler
  - normalization: o evacuated to SBUF, den row pair reciprocal'd
    partition-packed ([128,8]) via a DRAM round trip on the gpsimd
    ring, broadcast, multiplied; deferred into the next unit's stream
  - tail: hp0-2 partials of the last q-block pre-accumulated during
    hp3/qc0; fused finish runs per-head K=64 matmul pairs off the
    UNNORMALIZED last o straight from PSUM and rescales by
    partition-packed 1/den via scalar_tensor_tensor (mult+add with the
    pre-partial) -- no normalization round trip on the critical path
Matmul inputs bf16 (fp32 PSUM accumulation), softmax in fp32/bf16.
PSUM budget: scores 2x2 banks + o pair 2 + filler 2 = 8.
"""

import numpy as np

B, S, T, E, H, D = 4, 2048, 2048, 1024, 16, 64
N_CORES = 8
N_HP = 4  # head-pairs per core (8 heads)

_compiled = {}


def _mybir():
    from concourse import mybir

    return mybir


def build_program(n_hp=N_HP, s_loc=2048, t_len=2048, n_et=8, use_pairs=True, k0_early=True, warmup=True):
    import concourse.tile as tile
    from concourse import bacc

    mybir = _mybir()
    dt = mybir.dt
    bf16 = dt.bfloat16
    f32 = dt.float32
    ADD = mybir.AluOpType.add

    e_dim = 128 * n_et
    c_dim = 128 * n_hp  # concat dim on this core (512)
    n_h = 2 * n_hp
    n_tt = t_len // 128
    qch = 512
    n_qc = s_loc // qch
    tch = 512
    n_tc = t_len // tch

    nc = bacc.Bacc("TRN2", target_bir_lowering=False, debug=False)

    xT = nc.dram_tensor("xT", [128, n_et, s_loc], bf16, kind="ExternalInput").ap()
    yT = nc.dram_tensor("yT", [128, n_et, t_len], bf16, kind="ExternalInput").ap()
    wq2 = nc.dram_tensor("wq2", [n_hp, 128, n_et, 128], bf16, kind="ExternalInput").ap()
    wk2 = nc.dram_tensor("wk2", [n_hp, 128, n_et, 128], bf16, kind="ExternalInput").ap()
    wv = nc.dram_tensor("wv", [128, n_et, c_dim], bf16, kind="ExternalInput").ap()
    wo = nc.dram_tensor("wo", [128, n_hp, e_dim], bf16, kind="ExternalInput").ap()
    bqc = nc.dram_tensor("bqc", [128, n_hp], f32, kind="ExternalInput").ap()
    out = nc.dram_tensor("out", [s_loc, e_dim], bf16, kind="ExternalOutput").ap()

    from contextlib import ExitStack

    with tile.TileContext(nc) as tc, ExitStack() as ctx:
        consts = ctx.enter_context(tc.tile_pool(name="consts", bufs=1))
        qt_pool = ctx.enter_context(tc.tile_pool(name="qt", bufs=2))
        kt_pool = ctx.enter_context(tc.tile_pool(name="kt", bufs=2))
        exp_pool = ctx.enter_context(tc.tile_pool(name="expp", bufs=3))
        osc_pool = ctx.enter_context(tc.tile_pool(name="osc", bufs=6))
        rbc_pool = ctx.enter_context(tc.tile_pool(name="rbc", bufs=2))
        osb_pool = ctx.enter_context(tc.tile_pool(name="osb", bufs=3))
        tail_pool = ctx.enter_context(tc.tile_pool(name="tailp", bufs=1))
        scr_pool = ctx.enter_context(tc.tile_pool(name="scr", bufs=4, space="DRAM"))

        # ---- resident SBUF tensors ----
        xT_sb = consts.tile([128, n_et, s_loc], bf16)
        yT_sb = consts.tile([128, n_et, t_len], bf16)
        wq_sb = consts.tile([128, n_hp, n_et, 128], bf16)
        wk_sb = consts.tile([128, n_hp, n_et, 128], bf16)
        wv_sb = consts.tile([128, n_et, c_dim], bf16)
        wo_sb = consts.tile([128, n_hp, e_dim], bf16)
        bqc_sb = consts.tile([128, n_hp], f32)
        ones_row = consts.tile([1, 512], bf16)
        v_sb = consts.tile([128, n_tt, n_h, 65], bf16)
        oT_all = consts.tile([128, n_hp, s_loc], bf16)

        nc.vector.memset(ones_row, 1.0)
        nc.vector.memset(v_sb[:, :, :, 64:65], 1.0)

        # ---- DMA: y has priority on all three rings (V + K0 need it
        # first), wv right after, then x, then remaining weights ----
        # sync+gpsimd rings only: DMA descriptor issue on the scalar
        # engine (= ACT) would block the exp stream. t-chunk-major y so
        # K0(tc)/V(tt) start on the first quarter of y.
        rg = [nc.sync, nc.gpsimd]
        nc.sync.dma_start(out=wk_sb[:, 0, :, :], in_=wk2[0])
        for et in range(n_et):
            rg[et % 2].dma_start(
                out=yT_sb[:, et, 0:tch], in_=yT[:, et, 0:tch]
            )
        nc.gpsimd.dma_start(out=wq_sb[:, 0, :, :], in_=wq2[0])
        nc.gpsimd.dma_start(out=bqc_sb, in_=bqc)
        for et in range(n_et):
            rg[(et + 1) % 2].dma_start(out=wv_sb[:, et, :], in_=wv[:, et, :])
        for tc_ in range(1, n_tc):
            for et in range(n_et):
                rg[(et + tc_) % 2].dma_start(
                    out=yT_sb[:, et, tc_ * tch : (tc_ + 1) * tch],
                    in_=yT[:, et, tc_ * tch : (tc_ + 1) * tch],
                )
        for et in range(n_et):
            rg[et % 2].dma_start(
                out=xT_sb[:, et, 0:tch], in_=xT[:, et, 0:tch]
            )
        for et in range(n_et):
            rg[(et + 1) % 2].dma_start(
                out=xT_sb[:, et, tch:s_loc], in_=xT[:, et, tch:s_loc]
            )
        for hp_ in range(1, n_hp):
            rg[hp_ % 2].dma_start(out=wk_sb[:, hp_, :, :], in_=wk2[hp_])
            rg[(hp_ + 1) % 2].dma_start(out=wq_sb[:, hp_, :, :], in_=wq2[hp_])

        # ---- PE warmup: tiny matmuls while the first DMAs land ----
        with tc.tile_pool(name="warm", bufs=1, space="PSUM") as warm_pool:
            wps = warm_pool.tile([128, 128], f32)
            for _ in range(40 if warmup else 0):
                nc.tensor.matmul(
                    out=wps,
                    lhsT=ones_row[0:1, 0:128],
                    rhs=ones_row[0:1, 0:128],
                    start=True,
                    stop=True,
                )

        # ---- K(hp0): per-tc accumulation as y t-chunks land; V-lead
        # interleaved (kacc scoped so PSUM peaks at warm+fill+kacc) ----
        kt0 = kt_pool.tile([128, t_len], bf16, tag="kt")

        fill_ps = ctx.enter_context(tc.tile_pool(name="fillps", bufs=2, space="PSUM"))
        pair_ps = {"pool": fill_ps}

        def k0_tc(tc_):
            ps = pair_ps["pool"].tile([128, tch], f32, tag="pa", name="k0ps")
            for et in range(n_et):
                nc.tensor.matmul(
                    out=ps,
                    lhsT=wk_sb[:, 0, et, :],
                    rhs=yT_sb[:, et, tc_ * tch : (tc_ + 1) * tch],
                    start=(et == 0),
                    stop=(et == n_et - 1),
                )
            nc.vector.tensor_copy(out=kt0[:, tc_ * tch : (tc_ + 1) * tch], in_=ps)

        def proj_chunk(lhs_w, rhs_x, width, out_sb, bias=None, n_k=n_et):
            """Col-split projection: per k-step two M=64 matmuls on PE col
            groups {0,1}/{2,3} (small LDWEIGHTS, hidden under streaming)
            accumulating into one PSUM bank; single-op evacuation."""
            pool = pair_ps["pool"]
            ps = pool.tile([128, width], f32, tag="pa")
            for k in range(n_k):
                nc.tensor.matmul(
                    out=ps,
                    lhsT=lhs_w(k),
                    rhs=rhs_x(k),
                    start=(k == 0),
                    stop=(k == n_k - 1),
                )
            if bias is not None:
                nc.vector.tensor_scalar_add(out=out_sb, in0=ps, scalar1=bias)
            else:
                nc.vector.tensor_copy(out=out_sb, in_=ps)

        # ---- lead scope: V starts at y-done (x still landing), Q0
        # chunks interleaved into the V stream as x arrives ----
        qt0 = qt_pool.tile([128, s_loc], bf16, tag="qt")
        def q0_chunk(sc_):
            proj_chunk(
                lambda et: wq_sb[:, 0, et, :],
                lambda et: xT_sb[:, et, sc_ * qch : (sc_ + 1) * qch],
                qch,
                qt0[:, sc_ * qch : (sc_ + 1) * qch],
                bias=bqc_sb[:, 0:1],
            )

        vbox = {}

        def v_half(tt_, half):
            """Half a V tile (4 of 8 K-steps) per unit-1 slot: spreads the
            8-MM burst so ACT idles ~0.4us/slot instead of ~1.5."""
            if half == 0:
                vbox[tt_] = pair_ps["pool"].tile(
                    [128, c_dim], f32, tag="pa", name="vps"
                )
                for et in range(4):
                    nc.tensor.matmul(
                        out=vbox[tt_],
                        lhsT=yT_sb[:, et, tt_ * 128 : (tt_ + 1) * 128],
                        rhs=wv_sb[:, et, :],
                        start=(et == 0),
                        stop=False,
                    )
            else:
                ps = vbox.pop(tt_)
                for et in range(4, n_et):
                    nc.tensor.matmul(
                        out=ps,
                        lhsT=yT_sb[:, et, tt_ * 128 : (tt_ + 1) * 128],
                        rhs=wv_sb[:, et, :],
                        start=False,
                        stop=(et == n_et - 1),
                    )
                nc.vector.tensor_copy(
                    out=v_sb[:, tt_, :, 0:64],
                    in_=ps.rearrange("p (h d) -> p h d", d=64),
                )

        def gen_q0(sc_):
            box = {}
            for k in range(n_et):
                def _mm(k=k, box=box):
                    if k == 0:
                        box["ps"] = pair_ps["pool"].tile(
                            [128, qch], f32, tag="pa", name="q0f"
                        )
                    nc.tensor.matmul(
                        out=box["ps"],
                        lhsT=wq_sb[:, 0, k, :],
                        rhs=xT_sb[:, k, sc_ * qch : (sc_ + 1) * qch],
                        start=(k == 0),
                        stop=(k == n_et - 1),
                    )
                yield _mm
            def _ev(box=box):
                nc.vector.tensor_scalar_add(
                    out=qt0[:, sc_ * qch : (sc_ + 1) * qch],
                    in0=box["ps"],
                    scalar1=bqc_sb[:, 0:1],
                )
            yield _ev

        def v_proj_tt(tt):
            # V chunk [t-tile, c] with ones column via a K=1 bias matmul
            pool = pair_ps["pool"]
            ps = pool.tile([128, c_dim], f32, tag="pa", name="vps")
            for et in range(n_et):
                nc.tensor.matmul(
                    out=ps,
                    lhsT=yT_sb[:, et, tt * 128 : (tt + 1) * 128],
                    rhs=wv_sb[:, et, :],
                    start=(et == 0),
                    stop=(et == n_et - 1),
                )
            nc.vector.tensor_copy(
                out=v_sb[:, tt, :, 0:64],
                in_=ps.rearrange("p (h d) -> p h d", d=64),
            )

        # ---- filler: Q/K(hp1..) + out-proj interleaved into units ----
        qt_tiles = {0: qt0}
        kt_tiles = {0: kt0}

        def gen_qk(hp):
            qt = qt_pool.tile([128, s_loc], bf16, tag="qt")
            kt = kt_pool.tile([128, t_len], bf16, tag="kt")
            qt_tiles[hp] = qt
            kt_tiles[hp] = kt
            for kind in ("k", "q"):
                w_sb, x_sb, o_t, n_c = (
                    (wk_sb, yT_sb, kt, n_tc)
                    if kind == "k"
                    else (wq_sb, xT_sb, qt, n_qc)
                )
                for c_ in range(n_c):
                    box = {}
                    for k in range(n_et):
                        def _mm(k=k, c_=c_, box=box, w_sb=w_sb, x_sb=x_sb):
                            if k == 0:
                                box["ps"] = pair_ps["pool"].tile(
                                    [128, 512], f32, tag="pa", name="fps"
                                )
                            nc.tensor.matmul(
                                out=box["ps"],
                                lhsT=w_sb[:, hp, k, :],
                                rhs=x_sb[:, k, c_ * 512 : (c_ + 1) * 512],
                                start=(k == 0),
                                stop=(k == n_et - 1),
                            )
                        yield _mm
                    def _ev(c_=c_, box=box, o_t=o_t, kind=kind):
                        if kind == "q":
                            nc.vector.tensor_scalar_add(
                                out=o_t[:, c_ * 512 : (c_ + 1) * 512],
                                in0=box["ps"],
                                scalar1=bqc_sb[:, hp : hp + 1],
                            )
                        else:
                            nc.vector.tensor_copy(
                                out=o_t[:, c_ * 512 : (c_ + 1) * 512], in_=box["ps"]
                            )
                    yield _ev

        n_st_qc = qch // 128
        n_ec = 2
        ech = e_dim // n_ec

        def gen_outproj(qc):
            for st_ in range(n_st_qc):
                st = qc * n_st_qc + st_
                for ec_ in range(n_ec):
                    box = {}
                    for ct in range(n_hp):
                        def _mm(ct=ct, st=st, ec_=ec_, box=box):
                            if ct == 0:
                                box["ps"] = pair_ps["pool"].tile(
                                    [128, ech], f32, tag="pa", name="ops"
                                )
                            nc.tensor.matmul(
                                out=box["ps"],
                                lhsT=oT_all[:, ct, st * 128 : (st + 1) * 128],
                                rhs=wo_sb[:, ct, ec_ * ech : (ec_ + 1) * ech],
                                start=(ct == 0),
                                stop=(ct == n_hp - 1),
                            )
                        yield _mm
                    def _fin(st=st, ec_=ec_, box=box):
                        o_sb = osb_pool.tile([128, ech], bf16, tag="osb")
                        nc.vector.tensor_copy(out=o_sb, in_=box["ps"])
                        nc.sync.dma_start(
                            out=out[
                                st * 128 : (st + 1) * 128,
                                ec_ * ech : (ec_ + 1) * ech,
                            ],
                            in_=o_sb,
                        )
                    yield _fin

        oproj_parts = {}
        osb2_pool = ctx.enter_context(tc.tile_pool(name="osb2", bufs=8))

        def gen_outproj_pre(qc):
            for st_ in range(n_st_qc):
                st = qc * n_st_qc + st_
                for ec_ in range(n_ec):
                    box = {}
                    for ct in range(n_hp - 1):
                        def _mm(ct=ct, st=st, ec_=ec_, box=box):
                            if ct == 0:
                                box["ps"] = pair_ps["pool"].tile(
                                    [128, ech], f32, tag="pa", name="prps"
                                )
                            nc.tensor.matmul(
                                out=box["ps"],
                                lhsT=oT_all[:, ct, st * 128 : (st + 1) * 128],
                                rhs=wo_sb[:, ct, ec_ * ech : (ec_ + 1) * ech],
                                start=(ct == 0),
                                stop=(ct == n_hp - 2),
                            )
                        yield _mm
                    def _ev(st=st, ec_=ec_, box=box):
                        p_sb = osb2_pool.tile([128, ech], bf16, tag="part")
                        nc.vector.tensor_copy(out=p_sb, in_=box["ps"])
                        oproj_parts[(st, ec_)] = p_sb
                    yield _ev

        MUL = mybir.AluOpType.mult
        last_o = {}

        def tail_final(o_a, o_b):
            """hp3/qc3 finish without waiting for a normalization pass:
            per-head K=64 matmuls off the UNNORMALIZED o (row pairs
            co-execute), then a fused (ps*1/den)+part rescale-add with
            partition-packed reciprocals; engines alternate per block."""
            qc = n_qc - 1
            den_sb = tail_pool.tile([1, 2, qch], f32, tag="den", name="densb")
            nc.vector.tensor_copy(out=den_sb[0:1, 0, :], in_=o_a[64:65, :])
            nc.vector.tensor_copy(out=den_sb[0:1, 1, :], in_=o_b[64:65, :])
            osc_ab = tail_pool.tile([128, qch], bf16, tag="oscb", name="oscab")
            nc.vector.tensor_copy(out=osc_ab[0:64, :], in_=o_a[0:64, :])
            nc.vector.tensor_copy(out=osc_ab[64:128, :], in_=o_b[0:64, :])
            scr = scr_pool.tile([1, 2 * qch], f32, tag="scr")
            nc.scalar.dma_start(out=scr, in_=den_sb.rearrange("o h q -> o (h q)"))
            rca = tail_pool.tile([128, n_st_qc], f32, tag="rc", name="rca")
            nc.scalar.dma_start(
                out=rca, in_=scr[:, 0:qch].rearrange("o (i p) -> (o p) i", p=128)
            )
            rcb = tail_pool.tile([128, n_st_qc], f32, tag="rc2", name="rcb")
            nc.scalar.dma_start(
                out=rcb, in_=scr[:, qch:].rearrange("o (i p) -> (o p) i", p=128)
            )
            nc.vector.reciprocal(out=rca, in_=rca)
            nc.vector.reciprocal(out=rcb, in_=rcb)
            for st_ in range(n_st_qc):
                st = qc * n_st_qc + st_
                for ec_ in range(n_ec):
                    ps_a = pair_ps["pool"].tile([128, ech], f32, tag="pa", name="fpa")
                    ps_b = pair_ps["pool"].tile([128, ech], f32, tag="pa", name="fpb")
                    nc.tensor.matmul(
                        out=ps_a,
                        lhsT=osc_ab[0:64, st_ * 128 : (st_ + 1) * 128],
                        rhs=wo_sb[0:64, n_hp - 1, ec_ * ech : (ec_ + 1) * ech],
                        start=True,
                        stop=True,
                    )
                    nc.tensor.matmul(
                        out=ps_b,
                        lhsT=osc_ab[64:128, st_ * 128 : (st_ + 1) * 128],
                        rhs=wo_sb[64:128, n_hp - 1, ec_ * ech : (ec_ + 1) * ech],
                        start=True,
                        stop=True,
                    )
                    t1 = osb_pool.tile([128, ech], f32, tag="osb", name="t1")
                    nc.vector.scalar_tensor_tensor(
                        out=t1, in0=ps_a, scalar=rca[:, st_ : st_ + 1],
                        in1=oproj_parts[(st, ec_)], op0=MUL, op1=ADD,
                    )
                    o_sb = osb_pool.tile([128, ech], bf16, tag="osb", name="t2")
                    nc.vector.scalar_tensor_tensor(
                        out=o_sb, in0=ps_b, scalar=rcb[:, st_ : st_ + 1],
                        in1=t1, op0=MUL, op1=ADD,
                    )
                    ring = (nc.sync, nc.scalar, nc.gpsimd)[(2 * st_ + ec_) % 3]
                    ring.dma_start(
                        out=out[
                            st * 128 : (st + 1) * 128,
                            ec_ * ech : (ec_ + 1) * ech,
                        ],
                        in_=o_sb,
                    )

        filler = []

        def emit_filler(n):
            done = 0
            while filler and done < n:
                try:
                    task = next(filler[0])
                except StopIteration:
                    filler.pop(0)
                    continue
                task()
                done += 1

        def flush_filler_front():
            if not filler:
                return
            g = filler[0]
            while True:
                try:
                    task = next(g)
                except StopIteration:
                    break
                task()
            if filler and filler[0] is g:
                filler.pop(0)

        # ---- attention units ----
        # PSUM: scores 2x2 banks + o_a/o_b 2 + filler 2 = 8 banks.
        sc_ps = ctx.enter_context(tc.tile_pool(name="scps", bufs=2, space="PSUM"))
        o_ps = ctx.enter_context(tc.tile_pool(name="ops", bufs=2, space="PSUM"))

        exp_fn = mybir.ActivationFunctionType.Exp

        def emit_sc_exp(qt, kt, qc, tt):
            sc_t = sc_ps.tile([128, 2, qch], f32, tag="sc")
            nc.tensor.matmul(
                out=sc_t[:, 0, :],
                lhsT=kt[0:64, tt * 128 : (tt + 1) * 128],
                rhs=qt[0:64, qc * qch : (qc + 1) * qch],
                start=True,
                stop=True,
            )
            nc.tensor.matmul(
                out=sc_t[:, 1, :],
                lhsT=kt[64:128, tt * 128 : (tt + 1) * 128],
                rhs=qt[64:128, qc * qch : (qc + 1) * qch],
                start=True,
                stop=True,
            )
            exp_t = exp_pool.tile([128, 2, qch], bf16, tag="exp")
            nc.scalar.activation(out=exp_t, in_=sc_t, func=exp_fn, scale=0.125)
            return exp_t

        peel_box = []

        n_vlead = min(10, n_tt)
        k0_tc(0)
        v_proj_tt(0)
        v_proj_tt(1)
        k0_tc(1)
        v_proj_tt(2)
        v_proj_tt(3)
        k0_tc(2)
        v_proj_tt(4)
        v_proj_tt(5)
        k0_tc(3)
        for _tt in range(6, n_vlead):
            v_proj_tt(_tt)
        q0_chunk(0)
        pending_norm = [None]

        for hp in range(n_hp):
            if hp == 1 or n_hp == 1:
                nc.sync.dma_start(out=wo_sb, in_=wo)
            if hp + 1 < n_hp:
                filler.append(gen_qk(hp + 1))
            qt, kt = qt_tiles[hp], kt_tiles[hp]
            for qc in range(n_qc):
                if hp == n_hp - 1:
                    if pending_norm[0] is not None:
                        pending_norm[0]()
                        pending_norm[0] = None
                    if qc > 0:
                        filler.append(gen_outproj(qc - 1))
                o_a = o_ps.tile([65, qch], f32, tag="o")
                o_b = o_ps.tile([65, qch], f32, tag="o")
                pend = []
                for tt in range(n_tt):
                    if tt == 2 and pending_norm[0] is not None:
                        pending_norm[0]()
                        pending_norm[0] = None
                    if hp == n_hp - 1 and qc == 0 and tt == 3:
                        # hp0-2 partials of the last q-block only need norms
                        # done by this unit's tt2: pre-accumulate early
                        filler.append(gen_outproj_pre(n_qc - 1))
                    if hp == 0 and qc == 0 and tt < 12 and n_vlead + tt // 2 < n_tt:
                        v_half(n_vlead + tt // 2, tt % 2)
                    elif hp == 0 and qc == 0 and tt == 12:
                        q0_chunk(1)
                        # Q0 c2/c3 drain at filler priority in units 2-3
                        filler.insert(0, gen_q0(3))
                        filler.insert(0, gen_q0(2))
                    else:
                        n_emit = 2
                        if hp == n_hp - 1 and qc > 0 and tt % 4 == 0:
                            n_emit = 3
                        emit_filler(n_emit)
                    if tt == 0 and peel_box:
                        exp_t = peel_box.pop()
                    else:
                        exp_t = emit_sc_exp(qt, kt, qc, tt)

                    def attn_pair(tt, exp_t):
                        first, last = tt == 0, tt == n_tt - 1
                        nc.tensor.matmul(
                            out=o_a,
                            lhsT=v_sb[:, tt, 2 * hp, :],
                            rhs=exp_t[:, 0, :],
                            start=first,
                            stop=last,
                        )
                        nc.tensor.matmul(
                            out=o_b,
                            lhsT=v_sb[:, tt, 2 * hp + 1, :],
                            rhs=exp_t[:, 1, :],
                            start=first,
                            stop=last,
                        )

                    # lag attnV two tts behind exp so the PE never parks
                    # at the FIFO head waiting for ACT
                    pend.append((tt, exp_t))
                    if len(pend) > 2:
                        attn_pair(*pend.pop(0))
                if qc + 1 < n_qc:
                    # peel the next unit's first sc/exp ahead of the attnV
                    # drain so ACT never idles at the qc boundary
                    peel_box.append(emit_sc_exp(qt, kt, qc + 1, 0))
                while pend:
                    attn_pair(*pend.pop(0))
                if hp == n_hp - 1 and qc == n_qc - 1:
                    # tail_final reads o_a/o_b straight from PSUM; no osc
                    # evacuation or deferred norm for the last unit
                    if pending_norm[0] is not None:
                        pending_norm[0]()
                    pending_norm[0] = None
                    last_o["ab"] = (o_a, o_b)
                    continue
                # evacuate o NOW (program-order WAR keeps the banks
                # safe); defer the SBUF-side normalization chain into the
                # next unit's stream so ACT keeps running at the boundary
                osc_a = osc_pool.tile([65, qch], f32, tag="osc")
                nc.vector.tensor_copy(out=osc_a, in_=o_a)
                osc_b = osc_pool.tile([65, qch], f32, tag="osc")
                nc.vector.tensor_copy(out=osc_b, in_=o_b)

                def norm(hp=hp, qc=qc, osc_a=osc_a, osc_b=osc_b):
                    scr = scr_pool.tile([1, 2 * qch], f32, tag="scr")
                    nc.gpsimd.dma_start(out=scr[:, 0:qch], in_=osc_a[64:65, :])
                    nc.gpsimd.dma_start(out=scr[:, qch:], in_=osc_b[64:65, :])
                    den_pt = osc_pool.tile([128, 2 * qch // 128], f32, tag="dpt")
                    nc.gpsimd.dma_start(
                        out=den_pt,
                        in_=scr.rearrange("o (p i) -> (o p) i", p=128),
                    )
                    nc.vector.reciprocal(out=den_pt, in_=den_pt)
                    scr2 = scr_pool.tile([2, qch], f32, tag="scr2")
                    nc.gpsimd.dma_start(
                        out=scr2.rearrange("h (p i) -> (h p) i", p=64), in_=den_pt
                    )
                    rbc_a = rbc_pool.tile([64, qch], f32, tag="rbc")
                    nc.gpsimd.dma_start(out=rbc_a, in_=scr2[0:1, :].to_broadcast([64, qch]))
                    rbc_b = rbc_pool.tile([64, qch], f32, tag="rbc")
                    nc.gpsimd.dma_start(out=rbc_b, in_=scr2[1:2, :].to_broadcast([64, qch]))
                    nc.vector.tensor_mul(
                        oT_all[0:64, hp, qc * qch : (qc + 1) * qch],
                        osc_a[0:64, :],
                        rbc_a,
                    )
                    nc.vector.tensor_mul(
                        oT_all[64:128, hp, qc * qch : (qc + 1) * qch],
                        osc_b[0:64, :],
                        rbc_b,
                    )

                if pending_norm[0] is not None:
                    pending_norm[0]()
                pending_norm[0] = norm
            if hp + 1 < n_hp:
                flush_filler_front()

        # ---- tail: flush leftover fillers, then the fused hp3 finish ----
        while filler:
            flush_filler_front()
        tail_final(*last_o["ab"])

    nc.compile()
    return nc


def _bf16(a):
    import ml_dtypes

    return np.ascontiguousarray(a).astype(ml_dtypes.bfloat16)


def host_prep_half(Wq, bq, Wk, Wv, Wo, half, n_hp=N_HP, n_et=8):
    """Pack this head-half's weights into the kernel DRAM layouts."""
    e_dim = 128 * n_et
    c_dim = 128 * n_hp
    h0 = 2 * n_hp * half

    def pack_pairs(W):
        Wr = W[h0 : h0 + 2 * n_hp].reshape(n_hp, 2, e_dim, D)
        arr = Wr.transpose(2, 0, 1, 3).reshape(e_dim, n_hp, 128)
        arr = arr.reshape(n_et, 128, n_hp, 128).transpose(2, 1, 0, 3)
        return np.ascontiguousarray(arr)  # [hp, p, et, m]

    bqc = np.ascontiguousarray(
        bq[h0 : h0 + 2 * n_hp]
        .reshape(n_hp, 2, 64)
        .transpose(1, 2, 0)
        .reshape(128, n_hp)
    ).astype(np.float32)

    wv_cat = Wv[h0 : h0 + 2 * n_hp].transpose(1, 0, 2).reshape(e_dim, c_dim)
    wv_arr = wv_cat.reshape(n_et, 128, c_dim).transpose(1, 0, 2)
    wo_arr = (
        Wo[h0 * D : (h0 + 2 * n_hp) * D].reshape(n_hp, 128, e_dim).transpose(1, 0, 2)
    )

    return {
        "wq2": _bf16(pack_pairs(Wq)),
        "wk2": _bf16(pack_pairs(Wk)),
        "wv": _bf16(np.ascontiguousarray(wv_arr)),
        "wo": _bf16(np.ascontiguousarray(wo_arr)),
        "bqc": bqc,
    }


def host_prep_xt(mat, n_et=8):
    """[rows, E] -> [p, et, rows] transposed tiled layout, bf16."""
    rows, e_dim = mat.shape
    assert e_dim == 128 * n_et
    arr = mat.T.reshape(n_et, 128, rows).transpose(1, 0, 2)
    return _bf16(arr)


def kernel(x, y, Wq, bq, Wk, bk, Wv, bv, Wo, bo):
    import os
    import sys

    if "/opt/trn_rl_repo" not in sys.path:
        sys.path.insert(0, "/opt/trn_rl_repo")
    from concourse import bass_utils

    x = np.asarray(x, dtype=np.float32)
    y = np.asarray(y, dtype=np.float32)

    if "prog" not in _compiled:
        _compiled["prog"] = build_program()
    nc = _compiled["prog"]

    Wq, bq, Wk, Wv, Wo = (
        np.asarray(a, np.float32) for a in (Wq, bq, Wk, Wv, Wo)
    )
    halves = [host_prep_half(Wq, bq, Wk, Wv, Wo, half=j) for j in range(2)]
    xT_b = [host_prep_xt(x[b]) for b in range(B)]
    yT_b = [host_prep_xt(y[b]) for b in range(B)]

    in_maps = []
    for c in range(N_CORES):
        b, j = c // 2, c % 2
        m = dict(halves[j])
        m["xT"] = xT_b[b]
        m["yT"] = yT_b[b]
        in_maps.append(m)

    trace = os.environ.get("TRN_ATTN_TRACE", "0") == "1"
    res = bass_utils.run_bass_kernel_spmd(
        nc, in_maps, core_ids=list(range(N_CORES)), trace=trace
    )
    _compiled["last_results"] = res
    # bias terms folded out of the kernel: bo, plus bv's exact
    # contribution bv @ Wo (softmax weights sum to 1); bk is a no-op.
    bo_eff = np.asarray(bo, np.float32) + np.asarray(bv, np.float32).reshape(
        H * D
    ) @ np.asarray(Wo, np.float32)
    out = np.empty((B, S, E), dtype=np.float32)
    for b in range(B):
        out[b] = (
            np.asarray(res.results[2 * b]["out"], np.float32)
            + np.asarray(res.results[2 * b + 1]["out"], np.float32)
            + bo_eff
        )
    return out


# revision 48
# speedup vs baseline: 1.0223x; 1.0223x over previous
"""Cross multi-head attention on 8 Trainium2 NeuronCores.

Sharding: tensor-parallel by heads x data-parallel by batch. Core c
handles batch b = c//2 and head-half j = c%2 (heads 8j..8j+7). Each core
computes its heads' Q/K/V over the full sequence, attention, and a
PARTIAL output projection (contraction over its 512 concat dims); the
host sums the two partials per batch and adds bo. No K/V recompute, no
collectives.

Per-core kernel (T-major layout; host pre-transposes x/y, packs weights
head-pair-interleaved and hp-outermost so every weight DMA is
contiguous). Optimizations over the 409us baseline (~395us now):
  - K bias dropped entirely (it only adds a per-q constant to scores ->
    softmax-invariant); V bias folded into host-side
    bo_eff = bo + bv @ Wo (softmax weights sum to 1, so the v-bias
    contributes exactly bv@Wo) -- kills the 16 K=1 bias matmuls
  - input DMA on the sync+gpsimd rings ONLY: DMA descriptor issue on
    the scalar engine (= ACT) would block the exp stream, which paces
    the whole kernel (256 x [128,1024] exp activations ~278us busy)
  - t-chunk-major y DMA + per-tc K(hp0) + interleaved V-lead so the PE
    starts dense, HAM-warm projection work on the first quarter of y;
    V 10..15, Q(hp0) c1..3 finish inside the first attention unit
  - per (head-pair, query-chunk) unit: scoresT = KT_h.T @ QT_h as K=64
    row pairs on complementary PE halves (co-execute, ~219ns/pair
    measured vs 434 serial); oT' += V'_h.T @ expT stays M=65 serial
    (64 d + a ones column accumulating the softmax denominator in PSUM
    row 64; 2x65 output columns cannot col-pair within 128 lanes)
  - attnV lags exp by TWO tts so the PE never parks at the FIFO head
    waiting on ACT; Q/K(hp+1) and out-projection matmuls are emitted
    between attention groups as rate-controlled filler
  - normalization: o evacuated to SBUF, den row pair reciprocal'd
    partition-packed ([128,8]) via a DRAM round trip on the gpsimd
    ring, broadcast, multiplied; deferred into the next unit's stream
  - tail: hp0-2 partials of the last q-block pre-accumulated during
    hp3/qc0; fused finish runs per-head K=64 matmul pairs off the
    UNNORMALIZED last o straight from PSUM and rescales by
    partition-packed 1/den via scalar_tensor_tensor (mult+add with the
    pre-partial) -- no normalization round trip on the critical path
Matmul inputs bf16 (fp32 PSUM accumulation), softmax in fp32/bf16.
PSUM budget: scores 2x2 banks + o pair 2 + filler 2 = 8.
"""

import numpy as np

B, S, T, E, H, D = 4, 2048, 2048, 1024, 16, 64
N_CORES = 8
N_HP = 4  # head-pairs per core (8 heads)

_compiled = {}


def _mybir():
    from concourse import mybir

    return mybir


def build_program(n_hp=N_HP, s_loc=2048, t_len=2048, n_et=8, use_pairs=True, k0_early=True, warmup=True):
    import concourse.tile as tile
    from concourse import bacc

    mybir = _mybir()
    dt = mybir.dt
    bf16 = dt.bfloat16
    f32 = dt.float32
    ADD = mybir.AluOpType.add

    e_dim = 128 * n_et
    c_dim = 128 * n_hp  # concat dim on this core (512)
    n_h = 2 * n_hp
    n_tt = t_len // 128
    qch = 512
    n_qc = s_loc // qch
    tch = 512
    n_tc = t_len // tch

    nc = bacc.Bacc("TRN2", target_bir_lowering=False, debug=False)

    xT = nc.dram_tensor("xT", [128, n_et, s_loc], bf16, kind="ExternalInput").ap()
    yT = nc.dram_tensor("yT", [128, n_et, t_len], bf16, kind="ExternalInput").ap()
    wq2 = nc.dram_tensor("wq2", [n_hp, 128, n_et, 128], bf16, kind="ExternalInput").ap()
    wk2 = nc.dram_tensor("wk2", [n_hp, 128, n_et, 128], bf16, kind="ExternalInput").ap()
    wv = nc.dram_tensor("wv", [128, n_et, c_dim], bf16, kind="ExternalInput").ap()
    wo = nc.dram_tensor("wo", [128, n_hp, e_dim], bf16, kind="ExternalInput").ap()
    bqc = nc.dram_tensor("bqc", [128, n_hp], f32, kind="ExternalInput").ap()
    out = nc.dram_tensor("out", [s_loc, e_dim], bf16, kind="ExternalOutput").ap()

    from contextlib import ExitStack

    with tile.TileContext(nc) as tc, ExitStack() as ctx:
        consts = ctx.enter_context(tc.tile_pool(name="consts", bufs=1))
        qt_pool = ctx.enter_context(tc.tile_pool(name="qt", bufs=2))
        kt_pool = ctx.enter_context(tc.tile_pool(name="kt", bufs=2))
        exp_pool = ctx.enter_context(tc.tile_pool(name="expp", bufs=3))
        osc_pool = ctx.enter_context(tc.tile_pool(name="osc", bufs=6))
        rbc_pool = ctx.enter_context(tc.tile_pool(name="rbc", bufs=2))
        osb_pool = ctx.enter_context(tc.tile_pool(name="osb", bufs=3))
        tail_pool = ctx.enter_context(tc.tile_pool(name="tailp", bufs=1))
        scr_pool = ctx.enter_context(tc.tile_pool(name="scr", bufs=4, space="DRAM"))

        # ---- resident SBUF tensors ----
        xT_sb = consts.tile([128, n_et, s_loc], bf16)
        yT_sb = consts.tile([128, n_et, t_len], bf16)
        wq_sb = consts.tile([128, n_hp, n_et, 128], bf16)
        wk_sb = consts.tile([128, n_hp, n_et, 128], bf16)
        wv_sb = consts.tile([128, n_et, c_dim], bf16)
        wo_sb = consts.tile([128, n_hp, e_dim], bf16)
        bqc_sb = consts.tile([128, n_hp], f32)
        ones_row = consts.tile([1, 512], bf16)
        v_sb = consts.tile([128, n_tt, n_h, 65], bf16)
        oT_all = consts.tile([128, n_hp, s_loc], bf16)

        nc.vector.memset(ones_row, 1.0)
        nc.vector.memset(v_sb[:, :, :, 64:65], 1.0)

        # ---- DMA: y has priority on all three rings (V + K0 need it
        # first), wv right after, then x, then remaining weights ----
        # sync+gpsimd rings only: DMA descriptor issue on the scalar
        # engine (= ACT) would block the exp stream. t-chunk-major y so
        # K0(tc)/V(tt) start on the first quarter of y.
        rg = [nc.sync, nc.gpsimd]
        nc.sync.dma_start(out=wk_sb[:, 0, :, :], in_=wk2[0])
        for et in range(n_et):
            rg[et % 2].dma_start(
                out=yT_sb[:, et, 0:tch], in_=yT[:, et, 0:tch]
            )
        nc.gpsimd.dma_start(out=wq_sb[:, 0, :, :], in_=wq2[0])
        nc.gpsimd.dma_start(out=bqc_sb, in_=bqc)
        for et in range(n_et):
            rg[(et + 1) % 2].dma_start(out=wv_sb[:, et, :], in_=wv[:, et, :])
        for tc_ in range(1, n_tc):
            for et in range(n_et):
                rg[(et + tc_) % 2].dma_start(
                    out=yT_sb[:, et, tc_ * tch : (tc_ + 1) * tch],
                    in_=yT[:, et, tc_ * tch : (tc_ + 1) * tch],
                )
        for et in range(n_et):
            rg[et % 2].dma_start(
                out=xT_sb[:, et, 0:tch], in_=xT[:, et, 0:tch]
            )
        for et in range(n_et):
            rg[(et + 1) % 2].dma_start(
                out=xT_sb[:, et, tch:s_loc], in_=xT[:, et, tch:s_loc]
            )
        for hp_ in range(1, n_hp):
            rg[hp_ % 2].dma_start(out=wk_sb[:, hp_, :, :], in_=wk2[hp_])
            rg[(hp_ + 1) % 2].dma_start(out=wq_sb[:, hp_, :, :], in_=wq2[hp_])

        # ---- PE warmup: tiny matmuls while the first DMAs land ----
        with tc.tile_pool(name="warm", bufs=1, space="PSUM") as warm_pool:
            wps = warm_pool.tile([128, 128], f32)
            for _ in range(40 if warmup else 0):
                nc.tensor.matmul(
                    out=wps,
                    lhsT=ones_row[0:1, 0:128],
                    rhs=ones_row[0:1, 0:128],
                    start=True,
                    stop=True,
                )

        # ---- K(hp0): per-tc accumulation as y t-chunks land; V-lead
        # interleaved (kacc scoped so PSUM peaks at warm+fill+kacc) ----
        kt0 = kt_pool.tile([128, t_len], bf16, tag="kt")

        fill_ps = ctx.enter_context(tc.tile_pool(name="fillps", bufs=2, space="PSUM"))
        pair_ps = {"pool": fill_ps}

        def k0_tc(tc_):
            ps = pair_ps["pool"].tile([128, tch], f32, tag="pa", name="k0ps")
            for et in range(n_et):
                nc.tensor.matmul(
                    out=ps,
                    lhsT=wk_sb[:, 0, et, :],
                    rhs=yT_sb[:, et, tc_ * tch : (tc_ + 1) * tch],
                    start=(et == 0),
                    stop=(et == n_et - 1),
                )
            nc.vector.tensor_copy(out=kt0[:, tc_ * tch : (tc_ + 1) * tch], in_=ps)

        def proj_chunk(lhs_w, rhs_x, width, out_sb, bias=None, n_k=n_et):
            """Col-split projection: per k-step two M=64 matmuls on PE col
            groups {0,1}/{2,3} (small LDWEIGHTS, hidden under streaming)
            accumulating into one PSUM bank; single-op evacuation."""
            pool = pair_ps["pool"]
            ps = pool.tile([128, width], f32, tag="pa")
            for k in range(n_k):
                nc.tensor.matmul(
                    out=ps,
                    lhsT=lhs_w(k),
                    rhs=rhs_x(k),
                    start=(k == 0),
                    stop=(k == n_k - 1),
                )
            if bias is not None:
                nc.vector.tensor_scalar_add(out=out_sb, in0=ps, scalar1=bias)
            else:
                nc.vector.tensor_copy(out=out_sb, in_=ps)

        # ---- lead scope: V starts at y-done (x still landing), Q0
        # chunks interleaved into the V stream as x arrives ----
        qt0 = qt_pool.tile([128, s_loc], bf16, tag="qt")
        def q0_chunk(sc_):
            proj_chunk(
                lambda et: wq_sb[:, 0, et, :],
                lambda et: xT_sb[:, et, sc_ * qch : (sc_ + 1) * qch],
                qch,
                qt0[:, sc_ * qch : (sc_ + 1) * qch],
                bias=bqc_sb[:, 0:1],
            )

        vbox = {}

        def v_half(tt_, half):
            """Half a V tile (4 of 8 K-steps) per unit-1 slot: spreads the
            8-MM burst so ACT idles ~0.4us/slot instead of ~1.5."""
            if half == 0:
                vbox[tt_] = pair_ps["pool"].tile(
                    [128, c_dim], f32, tag="pa", name="vps"
                )
                for et in range(4):
                    nc.tensor.matmul(
                        out=vbox[tt_],
                        lhsT=yT_sb[:, et, tt_ * 128 : (tt_ + 1) * 128],
                        rhs=wv_sb[:, et, :],
                        start=(et == 0),
                        stop=False,
                    )
            else:
                ps = vbox.pop(tt_)
                for et in range(4, n_et):
                    nc.tensor.matmul(
                        out=ps,
                        lhsT=yT_sb[:, et, tt_ * 128 : (tt_ + 1) * 128],
                        rhs=wv_sb[:, et, :],
                        start=False,
                        stop=(et == n_et - 1),
                    )
                nc.vector.tensor_copy(
                    out=v_sb[:, tt_, :, 0:64],
                    in_=ps.rearrange("p (h d) -> p h d", d=64),
                )

        def gen_q0(sc_):
            box = {}
            for k in range(n_et):
                def _mm(k=k, box=box):
                    if k == 0:
                        box["ps"] = pair_ps["pool"].tile(
                            [128, qch], f32, tag="pa", name="q0f"
                        )
                    nc.tensor.matmul(
                        out=box["ps"],
                        lhsT=wq_sb[:, 0, k, :],
                        rhs=xT_sb[:, k, sc_ * qch : (sc_ + 1) * qch],
                        start=(k == 0),
                        stop=(k == n_et - 1),
                    )
                yield _mm
            def _ev(box=box):
                nc.vector.tensor_scalar_add(
                    out=qt0[:, sc_ * qch : (sc_ + 1) * qch],
                    in0=box["ps"],
                    scalar1=bqc_sb[:, 0:1],
                )
            yield _ev

        def v_proj_tt(tt):
            # V chunk [t-tile, c] with ones column via a K=1 bias matmul
            pool = pair_ps["pool"]
            ps = pool.tile([128, c_dim], f32, tag="pa", name="vps")
            for et in range(n_et):
                nc.tensor.matmul(
                    out=ps,
                    lhsT=yT_sb[:, et, tt * 128 : (tt + 1) * 128],
                    rhs=wv_sb[:, et, :],
                    start=(et == 0),
                    stop=(et == n_et - 1),
                )
            nc.vector.tensor_copy(
                out=v_sb[:, tt, :, 0:64],
                in_=ps.rearrange("p (h d) -> p h d", d=64),
            )

        # ---- filler: Q/K(hp1..) + out-proj interleaved into units ----
        qt_tiles = {0: qt0}
        kt_tiles = {0: kt0}

        def gen_qk(hp):
            qt = qt_pool.tile([128, s_loc], bf16, tag="qt")
            kt = kt_pool.tile([128, t_len], bf16, tag="kt")
            qt_tiles[hp] = qt
            kt_tiles[hp] = kt
            for kind in ("k", "q"):
                w_sb, x_sb, o_t, n_c = (
                    (wk_sb, yT_sb, kt, n_tc)
                    if kind == "k"
                    else (wq_sb, xT_sb, qt, n_qc)
                )
                for c_ in range(n_c):
                    box = {}
                    for k in range(n_et):
                        def _mm(k=k, c_=c_, box=box, w_sb=w_sb, x_sb=x_sb):
                            if k == 0:
                                box["ps"] = pair_ps["pool"].tile(
                                    [128, 512], f32, tag="pa", name="fps"
                                )
                            nc.tensor.matmul(
                                out=box["ps"],
                                lhsT=w_sb[:, hp, k, :],
                                rhs=x_sb[:, k, c_ * 512 : (c_ + 1) * 512],
                                start=(k == 0),
                                stop=(k == n_et - 1),
                            )
                        yield _mm
                    def _ev(c_=c_, box=box, o_t=o_t, kind=kind):
                        if kind == "q":
                            nc.vector.tensor_scalar_add(
                                out=o_t[:, c_ * 512 : (c_ + 1) * 512],
                                in0=box["ps"],
                                scalar1=bqc_sb[:, hp : hp + 1],
                            )
                        else:
                            nc.vector.tensor_copy(
                                out=o_t[:, c_ * 512 : (c_ + 1) * 512], in_=box["ps"]
                            )
                    yield _ev

        n_st_qc = qch // 128
        n_ec = 2
        ech = e_dim // n_ec

        def gen_outproj(qc):
            for st_ in range(n_st_qc):
                st = qc * n_st_qc + st_
                for ec_ in range(n_ec):
                    box = {}
                    for ct in range(n_hp):
                        def _mm(ct=ct, st=st, ec_=ec_, box=box):
                            if ct == 0:
                                box["ps"] = pair_ps["pool"].tile(
                                    [128, ech], f32, tag="pa", name="ops"
                                )
                            nc.tensor.matmul(
                                out=box["ps"],
                                lhsT=oT_all[:, ct, st * 128 : (st + 1) * 128],
                                rhs=wo_sb[:, ct, ec_ * ech : (ec_ + 1) * ech],
                                start=(ct == 0),
                                stop=(ct == n_hp - 1),
                            )
                        yield _mm
                    def _fin(st=st, ec_=ec_, box=box):
                        o_sb = osb_pool.tile([128, ech], bf16, tag="osb")
                        nc.vector.tensor_copy(out=o_sb, in_=box["ps"])
                        nc.sync.dma_start(
                            out=out[
                                st * 128 : (st + 1) * 128,
                                ec_ * ech : (ec_ + 1) * ech,
                            ],
                            in_=o_sb,
                        )
                    yield _fin

        oproj_parts = {}
        osb2_pool = ctx.enter_context(tc.tile_pool(name="osb2", bufs=8))

        def gen_outproj_pre(qc):
            for st_ in range(n_st_qc):
                st = qc * n_st_qc + st_
                for ec_ in range(n_ec):
                    box = {}
                    for ct in range(n_hp - 1):
                        def _mm(ct=ct, st=st, ec_=ec_, box=box):
                            if ct == 0:
                                box["ps"] = pair_ps["pool"].tile(
                                    [128, ech], f32, tag="pa", name="prps"
                                )
                            nc.tensor.matmul(
                                out=box["ps"],
                                lhsT=oT_all[:, ct, st * 128 : (st + 1) * 128],
                                rhs=wo_sb[:, ct, ec_ * ech : (ec_ + 1) * ech],
                                start=(ct == 0),
                                stop=(ct == n_hp - 2),
                            )
                        yield _mm
                    def _ev(st=st, ec_=ec_, box=box):
                        p_sb = osb2_pool.tile([128, ech], bf16, tag="part")
                        nc.vector.tensor_copy(out=p_sb, in_=box["ps"])
                        oproj_parts[(st, ec_)] = p_sb
                    yield _ev

        MUL = mybir.AluOpType.mult
        last_o = {}

        def tail_final(o_a, o_b):
            """hp3/qc3 finish without waiting for a normalization pass:
            per-head K=64 matmuls off the UNNORMALIZED o (row pairs
            co-execute), then a fused (ps*1/den)+part rescale-add with
            partition-packed reciprocals; engines alternate per block."""
            qc = n_qc - 1
            den_sb = tail_pool.tile([1, 2, qch], f32, tag="den", name="densb")
            nc.vector.tensor_copy(out=den_sb[0:1, 0, :], in_=o_a[64:65, :])
            nc.vector.tensor_copy(out=den_sb[0:1, 1, :], in_=o_b[64:65, :])
            osc_ab = tail_pool.tile([128, qch], bf16, tag="oscb", name="oscab")
            nc.vector.tensor_copy(out=osc_ab[0:64, :], in_=o_a[0:64, :])
            nc.vector.tensor_copy(out=osc_ab[64:128, :], in_=o_b[0:64, :])
            scr = scr_pool.tile([1, 2 * qch], f32, tag="scr")
            nc.scalar.dma_start(out=scr, in_=den_sb.rearrange("o h q -> o (h q)"))
            rca = tail_pool.tile([128, n_st_qc], f32, tag="rc", name="rca")
            nc.scalar.dma_start(
                out=rca, in_=scr[:, 0:qch].rearrange("o (i p) -> (o p) i", p=128)
            )
            rcb = tail_pool.tile([128, n_st_qc], f32, tag="rc2", name="rcb")
            nc.scalar.dma_start(
                out=rcb, in_=scr[:, qch:].rearrange("o (i p) -> (o p) i", p=128)
            )
            nc.vector.reciprocal(out=rca, in_=rca)
            nc.vector.reciprocal(out=rcb, in_=rcb)
            for st_ in range(n_st_qc):
                st = qc * n_st_qc + st_
                for ec_ in range(n_ec):
                    ps_a = pair_ps["pool"].tile([128, ech], f32, tag="pa", name="fpa")
                    ps_b = pair_ps["pool"].tile([128, ech], f32, tag="pa", name="fpb")
                    nc.tensor.matmul(
                        out=ps_a,
                        lhsT=osc_ab[0:64, st_ * 128 : (st_ + 1) * 128],
                        rhs=wo_sb[0:64, n_hp - 1, ec_ * ech : (ec_ + 1) * ech],
                        start=True,
                        stop=True,
                    )
                    nc.tensor.matmul(
                        out=ps_b,
                        lhsT=osc_ab[64:128, st_ * 128 : (st_ + 1) * 128],
                        rhs=wo_sb[64:128, n_hp - 1, ec_ * ech : (ec_ + 1) * ech],
                        start=True,
                        stop=True,
                    )
                    t1 = osb_pool.tile([128, ech], f32, tag="osb", name="t1")
                    nc.vector.scalar_tensor_tensor(
                        out=t1, in0=ps_a, scalar=rca[:, st_ : st_ + 1],
                        in1=oproj_parts[(st, ec_)], op0=MUL, op1=ADD,
                    )
                    o_sb = osb_pool.tile([128, ech], bf16, tag="osb", name="t2")
                    nc.vector.scalar_tensor_tensor(
                        out=o_sb, in0=ps_b, scalar=rcb[:, st_ : st_ + 1],
                        in1=t1, op0=MUL, op1=ADD,
                    )
                    ring = (nc.sync, nc.scalar, nc.gpsimd)[(2 * st_ + ec_) % 3]
                    ring.dma_start(
                        out=out[
                            st * 128 : (st + 1) * 128,
                            ec_ * ech : (ec_ + 1) * ech,
                        ],
                        in_=o_sb,
                    )

        filler = []

        def emit_filler(n):
            done = 0
            while filler and done < n:
                try:
                    task = next(filler[0])
                except StopIteration:
                    filler.pop(0)
                    continue
                task()
                done += 1

        def flush_filler_front():
            if not filler:
                return
            g = filler[0]
            while True:
                try:
                    task = next(g)
                except StopIteration:
                    break
                task()
            if filler and filler[0] is g:
                filler.pop(0)

        # ---- attention units ----
        # PSUM: scores 2x2 banks + o_a/o_b 2 + filler 2 = 8 banks.
        sc_ps = ctx.enter_context(tc.tile_pool(name="scps", bufs=2, space="PSUM"))
        o_ps = ctx.enter_context(tc.tile_pool(name="ops", bufs=2, space="PSUM"))

        exp_fn = mybir.ActivationFunctionType.Exp

        n_vlead = min(10, n_tt)
        k0_tc(0)
        v_proj_tt(0)
        v_proj_tt(1)
        k0_tc(1)
        v_proj_tt(2)
        v_proj_tt(3)
        k0_tc(2)
        v_proj_tt(4)
        v_proj_tt(5)
        k0_tc(3)
        for _tt in range(6, n_vlead):
            v_proj_tt(_tt)
        q0_chunk(0)
        pending_norm = [None]

        for hp in range(n_hp):
            if hp == 1 or n_hp == 1:
                nc.sync.dma_start(out=wo_sb, in_=wo)
            if hp + 1 < n_hp:
                filler.append(gen_qk(hp + 1))
            qt, kt = qt_tiles[hp], kt_tiles[hp]
            for qc in range(n_qc):
                if hp == n_hp - 1:
                    if pending_norm[0] is not None:
                        pending_norm[0]()
                        pending_norm[0] = None
                    if qc > 0:
                        filler.append(gen_outproj(qc - 1))
                o_a = o_ps.tile([65, qch], f32, tag="o")
                o_b = o_ps.tile([65, qch], f32, tag="o")
                pend = []
                for tt in range(n_tt):
                    if tt == 2 and pending_norm[0] is not None:
                        pending_norm[0]()
                        pending_norm[0] = None
                    if hp == n_hp - 1 and qc == 0 and tt == 3:
                        # hp0-2 partials of the last q-block only need norms
                        # done by this unit's tt2: pre-accumulate early
                        filler.append(gen_outproj_pre(n_qc - 1))
                    if hp == 0 and qc == 0 and tt < 12 and n_vlead + tt // 2 < n_tt:
                        v_half(n_vlead + tt // 2, tt % 2)
                    elif hp == 0 and qc == 0 and tt == 12:
                        q0_chunk(1)
                        # Q0 c2/c3 drain at filler priority in units 2-3
                        filler.insert(0, gen_q0(3))
                        filler.insert(0, gen_q0(2))
                    else:
                        n_emit = 2
                        if hp == n_hp - 1 and qc > 0 and tt % 4 == 0:
                            n_emit = 3
                        emit_filler(n_emit)
                    sc_t = sc_ps.tile([128, 2, qch], f32, tag="sc")
                    nc.tensor.matmul(
                        out=sc_t[:, 0, :],
                        lhsT=kt[0:64, tt * 128 : (tt + 1) * 128],
                        rhs=qt[0:64, qc * qch : (qc + 1) * qch],
                        start=True,
                        stop=True,
                    )
                    nc.tensor.matmul(
                        out=sc_t[:, 1, :],
                        lhsT=kt[64:128, tt * 128 : (tt + 1) * 128],
                        rhs=qt[64:128, qc * qch : (qc + 1) * qch],
                        start=True,
                        stop=True,
                    )
                    exp_t = exp_pool.tile([128, 2, qch], bf16, tag="exp")
                    nc.scalar.activation(out=exp_t, in_=sc_t, func=exp_fn, scale=0.125)

                    def attn_pair(tt, exp_t):
                        first, last = tt == 0, tt == n_tt - 1
                        nc.tensor.matmul(
                            out=o_a,
                            lhsT=v_sb[:, tt, 2 * hp, :],
                            rhs=exp_t[:, 0, :],
                            start=first,
                            stop=last,
                        )
                        nc.tensor.matmul(
                            out=o_b,
                            lhsT=v_sb[:, tt, 2 * hp + 1, :],
                            rhs=exp_t[:, 1, :],
                            start=first,
                            stop=last,
                        )

                    # lag attnV two tts behind exp so the PE never parks
                    # at the FIFO head waiting for ACT
                    pend.append((tt, exp_t))
                    if len(pend) > 2:
                        attn_pair(*pend.pop(0))
                while pend:
                    attn_pair(*pend.pop(0))
                if hp == n_hp - 1 and qc == n_qc - 1:
                    # tail_final reads o_a/o_b straight from PSUM; no osc
                    # evacuation or deferred norm for the last unit
                    if pending_norm[0] is not None:
                        pending_norm[0]()
                    pending_norm[0] = None
                    last_o["ab"] = (o_a, o_b)
                    continue
                # evacuate o NOW (program-order WAR keeps the banks
                # safe); defer the SBUF-side normalization chain into the
                # next unit's stream so ACT keeps running at the boundary
                osc_a = osc_pool.tile([65, qch], f32, tag="osc")
                nc.vector.tensor_copy(out=osc_a, in_=o_a)
                osc_b = osc_pool.tile([65, qch], f32, tag="osc")
                nc.vector.tensor_copy(out=osc_b, in_=o_b)

                def norm(hp=hp, qc=qc, osc_a=osc_a, osc_b=osc_b):
                    scr = scr_pool.tile([1, 2 * qch], f32, tag="scr")
                    nc.gpsimd.dma_start(out=scr[:, 0:qch], in_=osc_a[64:65, :])
                    nc.gpsimd.dma_start(out=scr[:, qch:], in_=osc_b[64:65, :])
                    den_pt = osc_pool.tile([128, 2 * qch // 128], f32, tag="dpt")
                    nc.gpsimd.dma_start(
                        out=den_pt,
                        in_=scr.rearrange("o (p i) -> (o p) i", p=128),
                    )
                    nc.vector.reciprocal(out=den_pt, in_=den_pt)
                    scr2 = scr_pool.tile([2, qch], f32, tag="scr2")
                    nc.gpsimd.dma_start(
                        out=scr2.rearrange("h (p i) -> (h p) i", p=64), in_=den_pt
                    )
                    rbc_a = rbc_pool.tile([64, qch], f32, tag="rbc")
                    nc.gpsimd.dma_start(out=rbc_a, in_=scr2[0:1, :].to_broadcast([64, qch]))
                    rbc_b = rbc_pool.tile([64, qch], f32, tag="rbc")
                    nc.gpsimd.dma_start(out=rbc_b, in_=scr2[1:2, :].to_broadcast([64, qch]))
                    nc.vector.tensor_mul(
                        oT_all[0:64, hp, qc * qch : (qc + 1) * qch],
                        osc_a[0:64, :],
                        rbc_a,
                    )
                    nc.vector.tensor_mul(
                        oT_all[64:128, hp, qc * qch : (qc + 1) * qch],
                        osc_b[0:64, :],
                        rbc_b,
                    )

                if pending_norm[0] is not None:
                    pending_norm[0]()
                pending_norm[0] = norm
            if hp + 1 < n_hp:
                flush_filler_front()

        # ---- tail: flush leftover fillers, then the fused hp3 finish ----
        while filler:
            flush_filler_front()
        tail_final(*last_o["ab"])

    nc.compile()
    return nc


def _bf16(a):
    import ml_dtypes

    return np.ascontiguousarray(a).astype(ml_dtypes.bfloat16)


def host_prep_half(Wq, bq, Wk, Wv, Wo, half, n_hp=N_HP, n_et=8):
    """Pack this head-half's weights into the kernel DRAM layouts."""
    e_dim = 128 * n_et
    c_dim = 128 * n_hp
    h0 = 2 * n_hp * half

    def pack_pairs(W):
        Wr = W[h0 : h0 + 2 * n_hp].reshape(n_hp, 2, e_dim, D)
        arr = Wr.transpose(2, 0, 1, 3).reshape(e_dim, n_hp, 128)
        arr = arr.reshape(n_et, 128, n_hp, 128).transpose(2, 1, 0, 3)
        return np.ascontiguousarray(arr)  # [hp, p, et, m]

    bqc = np.ascontiguousarray(
        bq[h0 : h0 + 2 * n_hp]
        .reshape(n_hp, 2, 64)
        .transpose(1, 2, 0)
        .reshape(128, n_hp)
    ).astype(np.float32)

    wv_cat = Wv[h0 : h0 + 2 * n_hp].transpose(1, 0, 2).reshape(e_dim, c_dim)
    wv_arr = wv_cat.reshape(n_et, 128, c_dim).transpose(1, 0, 2)
    wo_arr = (
        Wo[h0 * D : (h0 + 2 * n_hp) * D].reshape(n_hp, 128, e_dim).transpose(1, 0, 2)
    )

    return {
        "wq2": _bf16(pack_pairs(Wq)),
        "wk2": _bf16(pack_pairs(Wk)),
        "wv": _bf16(np.ascontiguousarray(wv_arr)),
        "wo": _bf16(np.ascontiguousarray(wo_arr)),
        "bqc": bqc,
    }


def host_prep_xt(mat, n_et=8):
    """[rows, E] -> [p, et, rows] transposed tiled layout, bf16."""
    rows, e_dim = mat.shape
    assert e_dim == 128 * n_et
    arr = mat.T.reshape(n_et, 128, rows).transpose(1, 0, 2)
    return _bf16(arr)


def kernel(x, y, Wq, bq, Wk, bk, Wv, bv, Wo, bo):
    import os
    import sys

    if "/opt/trn_rl_repo" not in sys.path:
        sys.path.insert(0, "/opt/trn_rl_repo")
    from concourse import bass_utils

    x = np.asarray(x, dtype=np.float32)
    y = np.asarray(y, dtype=np.float32)

    if "prog" not in _compiled:
        _compiled["prog"] = build_program()
    nc = _compiled["prog"]

    Wq, bq, Wk, Wv, Wo = (
        np.asarray(a, np.float32) for a in (Wq, bq, Wk, Wv, Wo)
    )
    halves = [host_prep_half(Wq, bq, Wk, Wv, Wo, half=j) for j in range(2)]
    xT_b = [host_prep_xt(x[b]) for b in range(B)]
    yT_b = [host_prep_xt(y[b]) for b in range(B)]

    in_maps = []
    for c in range(N_CORES):
        b, j = c // 2, c % 2
        m = dict(halves[j])
        m["xT"] = xT_b[b]
        m["yT"] = yT_b[b]
        in_maps.append(m)

    trace = os.environ.get("TRN_ATTN_TRACE", "0") == "1"
    res = bass_utils.run_bass_kernel_spmd(
        nc, in_maps, core_ids=list(range(N_CORES)), trace=trace
    )
    _compiled["last_results"] = res
    # bias terms folded out of the kernel: bo, plus bv's exact
    # contribution bv @ Wo (softmax weights sum to 1); bk is a no-op.
    bo_eff = np.asarray(bo, np.float32) + np.asarray(bv, np.float32).reshape(
        H * D
    ) @ np.asarray(Wo, np.float32)
    out = np.empty((B, S, E), dtype=np.float32)
    for b in range(B):
        out[b] = (
            np.asarray(res.results[2 * b]["out"], np.float32)
            + np.asarray(res.results[2 * b + 1]["out"], np.float32)
            + bo_eff
        )
    return out
